# revision 58
# baseline (speedup 1.0000x reference)
"""Diagonal complex SSM (LRU-style scan) on 8 trn2 NeuronCores.

y[t,p,k] = Re( C @ s[t,:,k] ) + (D @ x[t,:,k])
s[t,n,k] = A[n,k] * s[t-1,n,k] + (B @ x[t,:,k])[n]     (complex, diagonal)

Strategy: shard K=32 across 8 cores (4 lanes each, no collectives), and
decimate time by M=4 so the sequential scan runs on the lattice t_L=4L+3
only (T'=1024 per lane).  The decimation folds into precomputed matmul
stationaries:
  U_L    = sum_d (diag(a^d) B) @ x[4L+3-d]          (lattice input)
  S_L    = a^4 S_{L-1} + U_L                         (lattice scan)
  y[4L+c]= Re(C diag(a^{c+1})) @ ReIm(S_{L-1})       (recovery)
         + sum_{d<=c} Re(C diag(a^d) B + [d==0]D) @ x[4L+c-d]   (conv)
The complex lattice scan is phase-linearized (a^4 = r^4 e^{i4th}):
S_L = e^{i phi_L} Z_L with phi_L = th*t_L, giving two REAL hardware
scans per lane with decay r^4, plus elementwise rotations by
host-precomputed bf16 cos/sin tables packed as 3 windows [s | c | -s]
([s|c] and [c|-s] are views; rotate-out computes [w_re | -w_im] with a
subtract and the sign is folded into the recovery stationaries).
Everything except the scan state is bf16 (4x matmul, 2x DVE).  All
elementwise work runs on the vector engine: offloading to gpsimd is a
net loss (its TT ops are ~4x slower AND the SBUF contention inflates
concurrent DVE ops ~2-4x).
"""

import numpy as np
import ml_dtypes

from concourse import bacc, mybir
from concourse.tile import TileContext
from concourse.bass_utils import run_bass_kernel_spmd

T, N, U, K, P = 4096, 256, 128, 32, 128
NCORES = 8
KL = K // NCORES          # k-lanes per core
M = 4                     # time decimation
Tp = T // M               # lattice length (1024)
TB = 512                  # lattice points per chunk
NT = Tp // TB             # chunks (2)
F32 = mybir.dt.float32
BF16 = mybir.dt.bfloat16
BF = ml_dtypes.bfloat16

_CACHE = {}


def _build():
    nc = bacc.Bacc("TRN2", target_bir_lowering=False, debug=False,
                   num_devices=NCORES)

    xb_d = nc.dram_tensor("xb", [U, KL, NT, M, TB], BF16,
                          kind="ExternalInput")
    tab_d = nc.dram_tensor("tab3", [N, KL, NT, 3, TB], BF16,
                           kind="ExternalInput")
    r4_d = nc.dram_tensor("r4", [128, 2 * KL], F32, kind="ExternalInput")
    # stationaries pre-laid in SBUF layout (one identity DMA each)
    Bst_d = nc.dram_tensor("Bst", [U, KL * 16 * 128], BF16,
                           kind="ExternalInput")
    Wst_d = nc.dram_tensor("Wst", [128, KL * 16 * 128], BF16,
                           kind="ExternalInput")
    Cst_d = nc.dram_tensor("Cst", [U, KL * M * 128], BF16,
                           kind="ExternalInput")
    yb_d = nc.dram_tensor("yb", [P, KL, NT, M, TB], F32,
                          kind="ExternalOutput")

    mult = mybir.AluOpType.mult
    add = mybir.AluOpType.add

    def b2(ap):
        # [128, TB] -> [128, 2, TB] stride-0 pair broadcast
        return ap.rearrange("p (one tb) -> p one tb",
                            one=1).broadcast_to([128, 2, TB])

    with TileContext(nc) as tc:
        with (
            tc.tile_pool(name="const", bufs=1) as cpool,
            tc.tile_pool(name="xp", bufs=3) as xpool,
            tc.tile_pool(name="tab", bufs=3) as tabpool,
            tc.tile_pool(name="u2", bufs=3) as u2pool,
            tc.tile_pool(name="uh", bufs=3) as uhpool,
            tc.tile_pool(name="sh", bufs=2) as shpool,
            tc.tile_pool(name="w", bufs=2) as wpool,
            tc.tile_pool(name="yo", bufs=3) as ypool,
            tc.tile_pool(name="ups", bufs=2, space="PSUM") as upsum,
            tc.tile_pool(name="yps", bufs=1, space="PSUM") as ypsum,
        ):
            # startup-latency-ordered loads: k=0's stationaries first, the
            # rest per-k behind them; W/C (needed one k later) on the
            # scalar queue.
            # startup priority: only what (tb=0,k=0) needs right away; the
            # bulk stationaries are emitted inside the first iteration so
            # they don't compete with the critical first tiles.
            Bsb = cpool.tile([U, KL * 16 * 128], BF16)
            r4sb = cpool.tile([128, 2 * KL], F32)
            nc.sync.dma_start(r4sb[:], r4_d[:])
            # k=0 stationaries split across two queues (per-queue DMA
            # bandwidth limits the cold-start transfer)
            nc.gpsimd.dma_start(Bsb[:, 0:8 * 128], Bst_d[:, 0:8 * 128])
            Wsb = cpool.tile([128, KL * 16 * 128], BF16)
            Csb = cpool.tile([U, KL * M * 128], BF16)

            def bslice(k, d, ri, h):
                i = ((k * 4 + d) * 2 + ri) * 2 + h
                return Bsb[:, i * 128:(i + 1) * 128]

            def wslice(k, c, ri, h):
                i = ((k * 4 + c) * 2 + ri) * 2 + h
                return Wsb[:, i * 128:(i + 1) * 128]

            def cslice(k, d):
                i = k * M + d
                return Csb[:, i * 128:(i + 1) * 128]

            def y_assembly(k, xt, wexts, tby, final=False):
                # y assembly per offset class c (runs one k behind the
                # scan pipeline so tensor B-matmuls never stall behind it)
                y_sb = ypool.tile([P, M, TB], F32, tag="ysb")
                for c in range(M):
                    y_ps = ypsum.tile([P, TB], F32, tag=f"y{c}")
                    nmm = 0
                    total = 4 + (c + 1)
                    # conv first: depends only on xt, so it can start while
                    # the recovery inputs (w) are still being computed
                    for d in range(c + 1):
                        nc.tensor.matmul(
                            y_ps[:], cslice(k, d), xt[:, c - d, :],
                            start=(nmm == 0), stop=(nmm == total - 1))
                        nmm += 1
                    for h in (0, 1):
                        for ri in (0, 1):
                            nc.tensor.matmul(
                                y_ps[:], wslice(k, c, ri, h),
                                wexts[h][:, ri, 1:TB + 1],
                                start=(nmm == 0), stop=(nmm == total - 1))
                            nmm += 1
                    nc.scalar.copy(y_sb[:, c, :], y_ps[:])
                    if final:
                        nc.scalar.dma_start(yb_d[:, k, tby, c, :],
                                            y_sb[:, c, :])
                if not final:
                    nc.scalar.dma_start(yb_d[:, k, tby, :, :], y_sb[:])

            prev = {}
            wprev = {}
            pending = None        # (k, xt, wexts, L0) awaiting y assembly
            for tb in range(NT):
                L0 = tb * TB
                for k in range(KL):
                    xt = xpool.tile([U, M, TB], BF16, tag="x")
                    if tb == 0 and k == 0:
                        # cold start: split the first x tile across queues
                        nc.sync.dma_start(xt[:, 0:2, :],
                                          xb_d[:, k, tb, 0:2, :])
                        nc.scalar.dma_start(xt[:, 2:4, :],
                                            xb_d[:, k, tb, 2:4, :])
                        nc.scalar.dma_start(Bsb[:, 8 * 128:16 * 128],
                                            Bst_d[:, 8 * 128:16 * 128])
                    else:
                        nc.sync.dma_start(xt[:], xb_d[:, k, tb, :, :])

                    stage = []   # per-h deferred rotate-out pieces
                    wexts = []
                    for h in (0, 1):
                        hs = slice(h * 128, (h + 1) * 128)
                        tab = tabpool.tile([128, 3, TB], BF16, tag=f"t{h}")
                        (nc.sync if h == 0 else nc.gpsimd).dma_start(
                            tab[:], tab_d[hs, k, tb, :, :])

                        # lattice-B: U_L = sum_d Bd @ x[:, M-1-d, :]
                        u_ps = upsum.tile([128, 2, TB], F32, tag="u")
                        for ri in (0, 1):
                            for d in range(M):
                                nc.tensor.matmul(u_ps[:, ri, :],
                                                 bslice(k, d, ri, h),
                                                 xt[:, M - 1 - d, :],
                                                 start=(d == 0),
                                                 stop=(d == M - 1))
                        u2 = u2pool.tile([128, 2, TB], BF16, tag="u2")
                        nc.scalar.copy(u2[:], u_ps[:])

                        # rotate-in: V = e^{-i phi} U
                        # pA = [c|-s]*[ure|ure], pB = [s|c]*[uim|uim]
                        # uh = pA+pB = [V_re | V_im]
                        pA = uhpool.tile([128, 2, TB], BF16, tag="pA")
                        nc.vector.tensor_mul(pA[:], tab[:, 1:3, :],
                                             b2(u2[:, 0, :]))
                        pB = uhpool.tile([128, 2, TB], BF16, tag="pB")
                        nc.vector.tensor_mul(pB[:], tab[:, 0:2, :],
                                             b2(u2[:, 1, :]))
                        uh = uhpool.tile([128, 2, TB], BF16, tag="uh")
                        nc.vector.tensor_add(uh[:], pA[:], pB[:])

                        # lattice scans: Z = scan(r^4, V)
                        ridx = h * KL + k
                        rb = r4sb[:, ridx:ridx + 1].broadcast_to([128, TB])
                        sh2 = shpool.tile([128, 2 * TB], BF16, tag=f"sh{k}{h}")
                        if tb == 0:
                            init_re, init_im = 0.0, 0.0
                        else:
                            pv = prev[(k, h)]
                            init_re = pv[:, TB - 1:TB]
                            init_im = pv[:, 2 * TB - 1:2 * TB]
                        nc.vector.tensor_tensor_scan(
                            sh2[:, 0:TB], rb, uh[:, 0, :], init_re, mult, add)
                        nc.vector.tensor_tensor_scan(
                            sh2[:, TB:2 * TB], rb, uh[:, 1, :], init_im,
                            mult, add)
                        prev[(k, h)] = sh2

                        # rotate-out pieces: launch gpsimd muls now, defer
                        # the DVE subtract until both halves' scans are
                        # emitted (avoids DVE head-of-line blocking on gp).
                        # w = [w_re | -w_im] = pAo - pBo; the -w_im sign is
                        # folded into the recovery stationaries (+Im(Wc)).
                        wext = wpool.tile([128, 2, TB + 2], BF16,
                                          tag=f"w{k}{h}")
                        if tb == 0:
                            nc.vector.memset(wext[:, :, 1:2], 0.0)
                        else:
                            nc.scalar.copy(wext[:, :, 1:2],
                                           wprev[(k, h)][:, :, TB + 1:TB + 2])
                        pAo = uhpool.tile([128, 2, TB], BF16, tag=f"pAo{h}")
                        nc.vector.tensor_mul(pAo[:], tab[:, 1:3, :],
                                             b2(sh2[:, 0:TB]))
                        pBo = uhpool.tile([128, 2, TB], BF16, tag=f"pBo{h}")
                        nc.vector.tensor_mul(pBo[:], tab[:, 0:2, :],
                                             b2(sh2[:, TB:2 * TB]))
                        stage.append((wext, pAo, pBo))
                        wexts.append(wext)
                        wprev[(k, h)] = wext

                    for wext, pAo, pBo in stage:
                        nc.vector.tensor_sub(wext[:, :, 2:TB + 2],
                                             pAo[:], pBo[:])

                    if tb == 0 and k == 0:
                        for kq in range(1, KL):
                            cs = slice(kq * 16 * 128, (kq + 1) * 16 * 128)
                            nc.gpsimd.dma_start(Bsb[:, cs], Bst_d[:, cs])
                        nc.scalar.dma_start(Wsb[:], Wst_d[:])
                        nc.scalar.dma_start(Csb[:], Cst_d[:])
                    if pending is not None:
                        y_assembly(*pending)
                    pending = (k, xt, wexts, tb)
            y_assembly(*pending, final=True)

    nc.compile()
    return nc


def _host_prep(input_sequence, A_re, A_im, B_re, B_im, C_re, C_im, D):
    """Build the per-core input maps (numpy only)."""
    x = np.ascontiguousarray(np.asarray(input_sequence, dtype=np.float32))
    A = (np.asarray(A_re, np.float64) + 1j * np.asarray(A_im, np.float64))
    Bm = (np.asarray(B_re, np.float64) + 1j * np.asarray(B_im, np.float64))
    Cm = (np.asarray(C_re, np.float64) + 1j * np.asarray(C_im, np.float64))
    Dm = np.asarray(D, np.float64)

    r = np.abs(A)                       # (N, K)
    th = np.angle(A)
    r4 = (r ** M).astype(np.float32)

    # lattice phase tables, 3 windows [s | c | -s]:
    # [s|c] at 0 (pB windows), [c|-s] at 1 (pA windows)
    tL = (M * np.arange(Tp) + M - 1).astype(np.float64)
    ang = (th[:, :, None] * tL[None, None, :]) % (2 * np.pi)  # (N, K, Tp)
    c_t = np.cos(ang)
    s_t = np.sin(ang)
    tab3 = np.stack([s_t, c_t, -s_t], axis=2).astype(BF)  # (N, K, 3, Tp)
    # chunk-major: (N, K, NT, 3, TB) so each DMA tile is row-contiguous
    tab3 = tab3.reshape(N, K, 3, NT, TB).transpose(0, 1, 3, 2, 4)

    # x blocked chunk-major: xb[u, k, tb, c, j] = x[4(tb*TB+j)+c, u, k]
    xb = (x.transpose(1, 2, 0).reshape(U, K, NT, TB, M)
          .transpose(0, 1, 2, 4, 3))
    xb = np.ascontiguousarray(xb).astype(BF)

    # stationaries per k
    dpow = np.arange(M)
    ak = A.T                                  # (K, N)
    Bd = (ak[:, None, :, None] ** dpow[None, :, None, None]) \
        * Bm[None, None, :, :]                # (K, M, N, U) = diag(a^d) B
    Wc = Cm[None, None, :, :] * (ak[:, None, None, :]
                                 ** np.arange(1, M + 1)[None, :, None, None])
    Dc = np.real(np.einsum('pn,kdn,nu->kdpu',
                           Cm, ak[:, None, :] ** dpow[None, :, None], Bm))
    Dc[:, 0] += Dm[None, :, :]

    in_maps = []
    for core in range(NCORES):
        ks = slice(core * KL, (core + 1) * KL)
        kk = range(core * KL, (core + 1) * KL)

        Bst = np.empty((U, KL * 16 * 128), np.float32)
        Wst = np.empty((128, KL * 16 * 128), np.float32)
        Cst = np.empty((U, KL * M * 128), np.float32)
        for ki, kg in enumerate(kk):
            for d in range(M):
                for ri in range(2):
                    part = np.real(Bd[kg, d]) if ri == 0 else np.imag(Bd[kg, d])
                    for h in range(2):
                        i = ((ki * 4 + d) * 2 + ri) * 2 + h
                        # lhsT [U, 128]: stat.T of rows h*128:(h+1)*128
                        Bst[:, i * 128:(i + 1) * 128] = \
                            part[h * 128:(h + 1) * 128, :].T
            for c in range(M):
                for ri in range(2):
                    # ri==1 consumes -w_im (w = pAo - pBo), so +Im here
                    part = np.real(Wc[kg, c]) if ri == 0 \
                        else np.imag(Wc[kg, c])
                    for h in range(2):
                        i = ((ki * 4 + c) * 2 + ri) * 2 + h
                        # lhsT [n-half, P]
                        Wst[:, i * 128:(i + 1) * 128] = \
                            part[:, h * 128:(h + 1) * 128].T
            for d in range(M):
                i = ki * M + d
                Cst[:, i * 128:(i + 1) * 128] = Dc[kg, d].T

        rc = r4[:, ks]                                   # (N, KL)
        rpk = np.concatenate([rc[:128, :], rc[128:, :]], axis=1)
        in_maps.append(dict(
            xb=np.ascontiguousarray(xb[:, ks]),
            tab3=np.ascontiguousarray(tab3[:, ks]),
            r4=np.ascontiguousarray(rpk, np.float32),
            Bst=Bst.astype(BF), Wst=Wst.astype(BF), Cst=Cst.astype(BF),
        ))
    return in_maps


def _get_nc():
    if "nc" not in _CACHE:
        _CACHE["nc"] = _build()
    return _CACHE["nc"]


def kernel(input_sequence, A_re, A_im, B_re, B_im, C_re, C_im, D,
           trace=False):
    nc = _get_nc()
    in_maps = _host_prep(input_sequence, A_re, A_im, B_re, B_im, C_re,
                         C_im, D)
    res = run_bass_kernel_spmd(nc, in_maps, core_ids=list(range(NCORES)),
                               trace=trace)
    out = np.empty((T, P, K), dtype=np.float32)
    for c in range(NCORES):
        yb = res.results[c]["yb"]                # (P, KL, NT, M, TB)
        # out[4(tb*TB+j)+m, p, k] = yb[p, k, tb, m, j]
        y = yb.transpose(2, 4, 3, 0, 1).reshape(T, P, KL)
        out[:, :, c * KL:(c + 1) * KL] = y
    if trace:
        _CACHE["exec_time_ns"] = res.exec_time_ns
    return out
